# revision 1
# baseline (speedup 1.0000x reference)
"""Trainium2 Bass kernel for nn_Chimera_80934363725826 (gnn_message_passing).

Math: the reference builds a grid-DAG adjacency A (left->right, top->bottom
edges, weights sigmoid(-(dt+bias)) * 0.95/sqrt(num_incident)), computes
M = (I-A)^{-1} by repeated squaring, and returns y = M @ x + D*x.

Since (I-A) is unit-lower-triangular in raster order with only two sub-
diagonals (-1 and -14), y = (I-A)^{-1} x is exactly the 2D first-order
recurrence
    y[i,j] = x[i,j] + al[i,j]*y[i,j-1] + at[i,j]*y[i-1,j]
over the 14x14 grid (per batch*head, per feature). We solve it with row-wise
prefix scans (tensor_tensor_scan) on the vector engine: 14 grid rows x two
feature-halves, batched over 96 local (b,h) pairs on the partition dim.

Layout trick: the host pre-transposes x per grid row to (row, feat, col) so
every on-chip operand of the serial chain is a flat contiguous slice (the
DVE pays ~4x for stride-64 inner access patterns; flat runs at 1 elem/cyc).
The host un-transposes the output. Scan coefficient vectors are fed as
stride-0 broadcast access patterns directly (no materialization).

Engine split (v3, from measured rates):
  - VectorE: coefficient products, per-row-half at-multiplies, the scans,
    and the fused D-skip (scalar_tensor_tensor) for the last row group.
  - GpSimd: the b = t + x adds of the serial chain (flat tensor_tensor)
    and the out = y + D*x adds for early row groups.
  - ScalarE: sigmoid edge weights, D*x scaling for early row groups.

Sharding: data-parallel over batch B=32 -> 4 batches/core on 8 cores.
"""

import numpy as np

import bass_rust
import concourse.bass as bass
import concourse.bacc as bacc
import concourse.mybir as mybir
from concourse.bass import AP
from concourse.tile import TileContext
from concourse.bass_utils import run_bass_kernel_spmd

F32 = mybir.dt.float32

HG, WG = 14, 14          # grid
L = HG * WG              # 196 nodes
B, NH, P = 32, 24, 64    # batch, heads, headdim
NCORES = 8
BLOC = B // NCORES       # 4 batches per core
NPART = BLOC * NH        # 96 partitions (b,h) pairs
ROW = P * WG             # 896 elements per grid row per partition
HALF = ROW // 2          # 448
FH = P // 2              # 32 features per half
TOT = L * P              # 12544 elements per partition
INVERSE_FACTOR = 0.95

# row groups for the out stage / DMA chunks (small last group so the final
# store transfer doesn't sit on the kernel tail)
OUT_GROUPS = [(0, 2), (3, 6), (7, 10), (11, 12), (13, 13)]
# which groups use the DVE fused stt path (the tail group: DVE is free then)
OUT_ON_DVE = {(11, 13)}
# feed scan coefficients as stride-0 broadcast APs (False: materialize via
# ScalarE broadcast copies — slower but CoreSim-executable)
RAW_SCAN = True
# add debug outputs (coeff + raw y) for HW bisection
DEBUG_OUTS = False

_CACHE = {}


def _host_tables():
    nie = 2.0 * np.ones((HG, WG))
    nie[:, 0] -= 1.0
    nie[0, :] -= 1.0
    nie[nie < 1e-6] = 1.0
    norm = (INVERSE_FACTOR / np.sqrt(nie)).astype(np.float32)
    mask_l = np.ones((HG, WG), np.float32)
    mask_l[:, 0] = 0.0
    mask_t = np.ones((HG, WG), np.float32)
    mask_t[0, :] = 0.0
    tab = np.concatenate([(norm * mask_l).ravel(), (norm * mask_t).ravel()])
    return tab.astype(np.float32)  # [392]


def _raw_scan(nc, out, data0, data1):
    """tensor_tensor_scan with multi-dim APs (state = data0*state + data1,
    chained in AP iteration order; our masked coefficients self-reset it).
    Bypasses the 2D-only assert in the bass wrapper; the walrus verifier
    accepts 2D/3D operands and the hardware runs it at scan line rate."""
    eng = nc.vector
    ins = mybir.InstTensorScalarPtr(
        name=nc.get_next_instruction_name(),
        is_tensor_tensor_scan=True,
        is_scalar_tensor_tensor=True,
        op0=mybir.AluOpType.mult,
        op1=mybir.AluOpType.add,
        ins=[eng.lower_ap(data0),
             eng.lower_ap_or_imm(0.0),
             eng.lower_ap(data1)],
        outs=[eng.lower_ap(out)],
    )
    return eng.add_instruction(ins)


def _build_program():
    nc = bacc.Bacc("TRN2", target_bir_lowering=False, debug=False,
                   num_devices=NCORES)
    xin = nc.dram_tensor("xin", [NPART, TOT], F32, kind="ExternalInput")
    # dt | table | -bias | D  packed in one tensor -> single input DMA
    mscin = nc.dram_tensor("mscin", [NPART, 4 * L + 2], F32,
                           kind="ExternalInput")
    yout = nc.dram_tensor("yout", [NPART, TOT], F32, kind="ExternalOutput")
    if DEBUG_OUTS:
        cdbg = nc.dram_tensor("cdbg", [NPART, 2 * L], F32, kind="ExternalOutput")
        ydbg = nc.dram_tensor("ydbg", [NPART, TOT], F32, kind="ExternalOutput")

    Copy = mybir.ActivationFunctionType.Copy

    with TileContext(nc) as tc:
        with tc.tile_pool(name="main", bufs=1) as pool, \
             tc.tile_pool(name="rowtmp", bufs=3) as rpool, \
             tc.tile_pool(name="stmp", bufs=1) as spool:
            xt = pool.tile([NPART, TOT], F32)     # x, (i, f, j) layout
            yt = pool.tile([NPART, TOT], F32)     # y, (i, f, j) layout
            ot = pool.tile([NPART, TOT], F32)     # out, (i, f, j) layout
            msc = pool.tile([NPART, 4 * L + 2], F32)
            coeff = pool.tile([NPART, 2 * L], F32)
            dtt = msc[:, 0:2 * L]
            tab = msc[:, 2 * L:4 * L]
            nb = msc[:, 4 * L:4 * L + 1]
            dp = msc[:, 4 * L + 1:4 * L + 2]

            # coefficient inputs first (sigmoid is the longest dep chain),
            # then the first two grid rows, then the rest of x
            nc.sync.dma_start(out=msc[:, :], in_=mscin[:, :])
            nc.sync.dma_start(out=xt[:, 0:2 * ROW], in_=xin[:, 0:2 * ROW])
            for r0, r1 in [(2, 4), (5, 8), (9, 13)]:
                nc.sync.dma_start(
                    out=xt[:, r0 * ROW:(r1 + 1) * ROW],
                    in_=xin[:, r0 * ROW:(r1 + 1) * ROW],
                )

            nc.scalar.activation(
                out=coeff[:, :], in_=dtt,
                func=mybir.ActivationFunctionType.Sigmoid,
                bias=nb, scale=-1.0,
            )
            nc.vector.tensor_mul(out=coeff[:, :], in0=coeff[:, :], in1=tab)

            def al_bc(i):
                return coeff[:, i * WG:(i + 1) * WG].unsqueeze(1) \
                    .broadcast_to([NPART, P, WG])

            def at_bc(i):
                return coeff[:, L + i * WG:L + (i + 1) * WG].unsqueeze(1) \
                    .broadcast_to([NPART, P, WG])

            def v3(t, off):
                """3D [NPART, P, WG] view of one row (896 elems) at off."""
                return t[:, off:off + ROW].rearrange(
                    "p (f j) -> p f j", f=P, j=WG)

            def jf(t, i):
                """strided (j, f)-order walk over row i of an (i,f,j) tile."""
                return t[:, i * ROW:(i + 1) * ROW].rearrange(
                    "p (f j) -> p j f", f=P, j=WG)

            def do_scan(i, data1_3d_ap):
                if RAW_SCAN:
                    _raw_scan(nc, v3(yt, i * ROW), al_bc(i), data1_3d_ap)
                else:
                    alx = rpool.tile([NPART, ROW], F32, tag="alx")
                    nc.scalar.activation(out=v3(alx, 0), in_=al_bc(i),
                                         func=Copy)
                    nc.vector.tensor_tensor_scan(
                        out=yt[:, i * ROW:(i + 1) * ROW], data0=alx[:, :],
                        data1=AP(data1_3d_ap.tensor, data1_3d_ap.offset,
                                 [data1_3d_ap.ap[0], [1, ROW]]),
                        initial=0.0, op0=mybir.AluOpType.mult,
                        op1=mybir.AluOpType.add)

            # --- row recurrence: all-DVE chain, full-row ops ---
            for i in range(HG):
                if i == 0:
                    do_scan(0, v3(xt, 0))
                else:
                    off = i * ROW
                    tt = rpool.tile([NPART, ROW], F32, tag="tt")
                    bt = rpool.tile([NPART, ROW], F32, tag="bt")
                    nc.vector.tensor_mul(
                        out=v3(tt, 0), in0=v3(yt, off - ROW), in1=at_bc(i))
                    nc.vector.tensor_add(
                        out=bt[:, :], in0=tt[:, :], in1=xt[:, off:off + ROW])
                    do_scan(i, v3(bt, 0))

                # out stage: early groups on ScalarE (D*x) + GpSimd adds in
                # deliberately strided order (address-bound, light on SBUF
                # ports so the DVE chain is not degraded); tail group fused
                # on the DVE once the chain is over.
                for g in OUT_GROUPS:
                    if i == g[1]:
                        o0, o1 = g[0] * ROW, (g[1] + 1) * ROW
                        nc.vector.scalar_tensor_tensor(
                            out=ot[:, o0:o1], in0=xt[:, o0:o1],
                            scalar=dp, in1=yt[:, o0:o1],
                            op0=mybir.AluOpType.mult,
                            op1=mybir.AluOpType.add)
                        nc.sync.dma_start(
                            out=yout[:, o0:o1], in_=ot[:, o0:o1])

            if DEBUG_OUTS:
                nc.sync.dma_start(out=cdbg[:, :], in_=coeff[:, :])
                nc.sync.dma_start(out=ydbg[:, :], in_=yt[:, :])

    nc.compile()
    return nc


def _get_program():
    if "nc" not in _CACHE:
        _CACHE["nc"] = _build_program()
    return _CACHE["nc"]


def make_in_maps(dt, dt_bias, x, D):
    """Host-side sharding + per-row (j,f)->(f,j) transpose + tables."""
    dt = np.ascontiguousarray(np.asarray(dt, dtype=np.float32))
    dt_bias = np.asarray(dt_bias, dtype=np.float32)
    x = np.asarray(x, dtype=np.float32)
    D = np.asarray(D, dtype=np.float32)

    tab = _host_tables()
    tabin = np.ascontiguousarray(np.broadcast_to(tab, (NPART, 2 * L)))
    nb = np.ascontiguousarray(np.tile(-dt_bias, BLOC).reshape(NPART, 1))
    dp = np.ascontiguousarray(np.tile(D, BLOC).reshape(NPART, 1))

    # pre-transpose x: (b,h,i,j,f) -> (b,h,i,f,j), flat per (b,h)
    xT = np.ascontiguousarray(
        x.reshape(B, NH, HG, WG, P).transpose(0, 1, 2, 4, 3)
        .reshape(B, NH, TOT))

    in_maps = []
    for c in range(NCORES):
        bs = slice(c * BLOC, (c + 1) * BLOC)
        dtc = np.ascontiguousarray(
            dt[:, bs].reshape(2, BLOC, NH, L).transpose(1, 2, 0, 3)
            .reshape(NPART, 2 * L))
        xc = np.ascontiguousarray(xT[bs].reshape(NPART, TOT))
        in_maps.append({
            "xin": xc,
            "mscin": np.ascontiguousarray(
                np.concatenate([dtc, tabin, nb, dp], axis=1)),
        })
    return in_maps


def _gather(results):
    """[96, TOT] (i,f,j)-layout shards -> full [B,NH,L,P]."""
    outs = []
    for r in results:
        o = r["yout"].reshape(BLOC, NH, HG, P, WG).transpose(0, 1, 2, 4, 3)
        outs.append(o.reshape(BLOC, NH, L, P))
    return np.ascontiguousarray(np.concatenate(outs, axis=0))


def kernel(dt, dt_bias, x, D):
    nc = _get_program()
    in_maps = make_in_maps(dt, dt_bias, x, D)
    res = run_bass_kernel_spmd(nc, in_maps, core_ids=list(range(NCORES)))
    return _gather(res.results)



# revision 2
# speedup vs baseline: 1.0578x; 1.0578x over previous
"""Trainium2 Bass kernel for nn_Chimera_80934363725826 (gnn_message_passing).

Math: the reference builds a grid-DAG adjacency A (left->right, top->bottom
edges, weights sigmoid(-(dt+bias)) * 0.95/sqrt(num_incident)), computes
M = (I-A)^{-1} by repeated squaring, and returns y = M @ x + D*x.

Since (I-A) is unit-lower-triangular in raster order with only two sub-
diagonals (-1 and -14), y = (I-A)^{-1} x is exactly the 2D first-order
recurrence
    y[i,j] = x[i,j] + al[i,j]*y[i,j-1] + at[i,j]*y[i-1,j]
over the 14x14 grid (per batch*head, per feature), solved with row-wise
prefix scans (tensor_tensor_scan) on the vector engine.

v4 (this file): fp16 data plane.
  - x is converted to fp16 on the host (rel err ~2.4e-4, gate is 2e-2) and
    shipped transposed to (row, feat, col); y and out are fp16 on device and
    converted back on the host. DMA bytes halve in both directions.
  - The serial chain stays on the DVE in ONE chain (scan -> mul -> add
    back-to-back on the same engine, no cross-engine latency): fp16 gets the
    DVE 2x_1p mode for mul/add (measured 515/536 ns per 896-elem row vs 1086
    fp32); the scan runs at 2 cyc/elem regardless of dtype (1931 ns).
  - The out = y + D*x stage moves entirely to the otherwise-idle GpSimd
    engine (measured ~2.2 ns/elem), off the DVE critical path. D==1 (the
    harness always generates D=ones) takes a fused y+x add; general D
    pre-scales x on the Scalar engine.
  - The Sigmoid activation table load (1283 ns) is pulled to t=0 via a dummy
    activation so it overlaps the input DMA ramp.

Sharding: data-parallel over batch B=32 -> 4 batches/core on 8 cores.
"""

import numpy as np

import concourse.bass as bass
import concourse.bacc as bacc
import concourse.mybir as mybir
from concourse.tile import TileContext
from concourse.bass_utils import run_bass_kernel_spmd

F32 = mybir.dt.float32
F16 = mybir.dt.float16

HG, WG = 14, 14          # grid
L = HG * WG              # 196 nodes
B, NH, P = 32, 24, 64    # batch, heads, headdim
NCORES = 8
BLOC = B // NCORES       # 4 batches per core
NPART = BLOC * NH        # 96 partitions (b,h) pairs
ROW = P * WG             # 896 elements per grid row per partition
TOT = L * P              # 12544 elements per partition
INVERSE_FACTOR = 0.95

# row groups for the out stage / output DMA chunks (small last group so the
# final store doesn't sit on the kernel tail)
OUT_GROUPS = [(0, 2), (3, 6), (7, 10), (11, 12), (13, 13)]
RAW_SCAN = True  # kept for test.py compat

_CACHE = {}


def _host_tables():
    nie = 2.0 * np.ones((HG, WG))
    nie[:, 0] -= 1.0
    nie[0, :] -= 1.0
    nie[nie < 1e-6] = 1.0
    norm = (INVERSE_FACTOR / np.sqrt(nie)).astype(np.float32)
    mask_l = np.ones((HG, WG), np.float32)
    mask_l[:, 0] = 0.0
    mask_t = np.ones((HG, WG), np.float32)
    mask_t[0, :] = 0.0
    tab = np.concatenate([(norm * mask_l).ravel(), (norm * mask_t).ravel()])
    return tab.astype(np.float32)  # [392]


def _raw_scan(nc, out, data0, data1):
    """tensor_tensor_scan with multi-dim APs (state = data0*state + data1,
    fp32 state, chained in AP iteration order; masked coefficients al[:,0]=0
    reset the chain at each feature boundary)."""
    eng = nc.vector
    ins = mybir.InstTensorScalarPtr(
        name=nc.get_next_instruction_name(),
        is_tensor_tensor_scan=True,
        is_scalar_tensor_tensor=True,
        op0=mybir.AluOpType.mult,
        op1=mybir.AluOpType.add,
        ins=[eng.lower_ap(data0),
             eng.lower_ap_or_imm(0.0),
             eng.lower_ap(data1)],
        outs=[eng.lower_ap(out)],
    )
    return eng.add_instruction(ins)


def _build_program(d_is_one=True):
    nc = bacc.Bacc("TRN2", target_bir_lowering=False, debug=False,
                   num_devices=NCORES)
    xin = nc.dram_tensor("xin", [NPART, TOT], F16, kind="ExternalInput")
    # dt | table | -bias | D  packed in one fp32 tensor -> single input DMA
    mscin = nc.dram_tensor("mscin", [NPART, 4 * L + 2], F32,
                           kind="ExternalInput")
    yout = nc.dram_tensor("yout", [NPART, TOT], F16, kind="ExternalOutput")

    Copy = mybir.ActivationFunctionType.Copy
    Sigmoid = mybir.ActivationFunctionType.Sigmoid

    with TileContext(nc) as tc:
        with tc.tile_pool(name="main", bufs=1) as pool, \
             tc.tile_pool(name="rowtmp", bufs=3) as rpool:
            xt = pool.tile([NPART, TOT], F16)     # x, (i, f, j) layout
            yt = pool.tile([NPART, TOT], F16)     # y
            ot = pool.tile([NPART, TOT], F16)     # out
            xd = pool.tile([NPART, TOT], F16) if not d_is_one else None
            msc = pool.tile([NPART, 4 * L + 2], F32)
            coeff = pool.tile([NPART, 2 * L], F32)
            c16 = pool.tile([NPART, 2 * L], F16)
            warm = pool.tile([NPART, 1], F32)
            dtt = msc[:, 0:2 * L]
            tab = msc[:, 2 * L:4 * L]
            nb = msc[:, 4 * L:4 * L + 1]
            dp = msc[:, 4 * L + 1:4 * L + 2]

            # pull the Sigmoid act-table load (1283ns) to t=0, before any
            # DMA lands
            nc.gpsimd.memset(warm[:, :], 0.0)
            nc.scalar.activation(out=warm[:, :], in_=warm[:, :], func=Sigmoid)

            # coefficient inputs first (sigmoid is the longest dep chain),
            # then the first rows of x, then the rest
            nc.sync.dma_start(out=msc[:, :], in_=mscin[:, :])
            nc.sync.dma_start(out=xt[:, 0:2 * ROW], in_=xin[:, 0:2 * ROW])
            for r0, r1 in [(2, 4), (5, 8), (9, 13)]:
                nc.sync.dma_start(
                    out=xt[:, r0 * ROW:(r1 + 1) * ROW],
                    in_=xin[:, r0 * ROW:(r1 + 1) * ROW],
                )

            nc.scalar.activation(out=coeff[:, :], in_=dtt, func=Sigmoid,
                                 bias=nb, scale=-1.0)
            # fold the normalization table in and downcast to fp16
            nc.vector.tensor_mul(out=c16[:, :], in0=coeff[:, :], in1=tab)

            if not d_is_one:
                # general-D path: xd = D*x on the Scalar engine (idle),
                # per out-group so it overlaps the chain
                for g0, g1 in OUT_GROUPS:
                    nc.scalar.activation(
                        out=xd[:, g0 * ROW:(g1 + 1) * ROW],
                        in_=xt[:, g0 * ROW:(g1 + 1) * ROW],
                        func=Copy, scale=dp)

            def al_bc(i):
                return c16[:, i * WG:(i + 1) * WG].unsqueeze(1) \
                    .broadcast_to([NPART, P, WG])

            def at_bc(i):
                return c16[:, L + i * WG:L + (i + 1) * WG].unsqueeze(1) \
                    .broadcast_to([NPART, P, WG])

            def v3(t, off):
                return t[:, off:off + ROW].rearrange(
                    "p (f j) -> p f j", f=P, j=WG)

            # --- row recurrence: single chain, all three ops on the DVE
            # back-to-back (no cross-engine latency); out-stage on GpSimd ---
            for i in range(HG):
                if i == 0:
                    _raw_scan(nc, v3(yt, 0), al_bc(0), v3(xt, 0))
                else:
                    off = i * ROW
                    tt = rpool.tile([NPART, ROW], F16, tag="tt")
                    vt = rpool.tile([NPART, ROW], F16, tag="vt")
                    nc.vector.tensor_mul(
                        out=v3(tt, 0), in0=v3(yt, off - ROW), in1=at_bc(i))
                    nc.vector.tensor_add(
                        out=vt[:, :], in0=tt[:, :], in1=xt[:, off:off + ROW])
                    _raw_scan(nc, v3(yt, off), al_bc(i), v3(vt, 0))

                for g in OUT_GROUPS:
                    if i == g[1]:
                        o0, o1 = g[0] * ROW, (g[1] + 1) * ROW
                        nc.gpsimd.tensor_add(
                            out=ot[:, o0:o1], in0=yt[:, o0:o1],
                            in1=(xt if d_is_one else xd)[:, o0:o1])
                        nc.sync.dma_start(
                            out=yout[:, o0:o1], in_=ot[:, o0:o1])

    nc.compile()
    return nc


def _get_program(d_is_one=True):
    key = ("nc", d_is_one)
    if key not in _CACHE:
        _CACHE[key] = _build_program(d_is_one)
    return _CACHE[key]


def make_in_maps(dt, dt_bias, x, D):
    """Host-side sharding + per-row (j,f)->(f,j) transpose + fp16 + tables."""
    dt = np.ascontiguousarray(np.asarray(dt, dtype=np.float32))
    dt_bias = np.asarray(dt_bias, dtype=np.float32)
    x = np.asarray(x, dtype=np.float32)
    D = np.asarray(D, dtype=np.float32)

    tab = _host_tables()
    tabin = np.ascontiguousarray(np.broadcast_to(tab, (NPART, 2 * L)))
    nb = np.ascontiguousarray(np.tile(-dt_bias, BLOC).reshape(NPART, 1))
    dp = np.ascontiguousarray(np.tile(D, BLOC).reshape(NPART, 1))

    # pre-transpose x: (b,h,i,j,f) -> (b,h,i,f,j), flat per (b,h), fp16
    xT = np.ascontiguousarray(
        x.reshape(B, NH, HG, WG, P).transpose(0, 1, 2, 4, 3)
        .reshape(B, NH, TOT)).astype(np.float16)

    in_maps = []
    for c in range(NCORES):
        bs = slice(c * BLOC, (c + 1) * BLOC)
        dtc = np.ascontiguousarray(
            dt[:, bs].reshape(2, BLOC, NH, L).transpose(1, 2, 0, 3)
            .reshape(NPART, 2 * L))
        xc = np.ascontiguousarray(xT[bs].reshape(NPART, TOT))
        in_maps.append({
            "xin": xc,
            "mscin": np.ascontiguousarray(
                np.concatenate([dtc, tabin, nb, dp], axis=1)),
        })
    return in_maps


def _gather(results):
    """[96, TOT] fp16 (i,f,j)-layout shards -> full fp32 [B,NH,L,P]."""
    outs = []
    for r in results:
        o = np.asarray(r["yout"], dtype=np.float32)
        o = o.reshape(BLOC, NH, HG, P, WG).transpose(0, 1, 2, 4, 3)
        outs.append(o.reshape(BLOC, NH, L, P))
    return np.ascontiguousarray(np.concatenate(outs, axis=0))


def kernel(dt, dt_bias, x, D):
    d_is_one = bool(np.allclose(np.asarray(D, np.float32), 1.0))
    nc = _get_program(d_is_one)
    in_maps = make_in_maps(dt, dt_bias, x, D)
    res = run_bass_kernel_spmd(nc, in_maps, core_ids=list(range(NCORES)))
    return _gather(res.results)


# revision 5
# speedup vs baseline: 1.3025x; 1.2313x over previous
"""Trainium2 Bass kernel for nn_Chimera_80934363725826 (gnn_message_passing).

Math: the reference builds a grid-DAG adjacency A (left->right, top->bottom
edges, weights sigmoid(-(dt+bias)) * 0.95/sqrt(num_incident)), computes
M = (I-A)^{-1} by repeated squaring, and returns y = M @ x + D*x.

Since (I-A) is unit-lower-triangular in raster order with only two sub-
diagonals (-1 and -14), y = (I-A)^{-1} x is exactly the 2D first-order
recurrence
    y[i,j] = x[i,j] + al[i,j]*y[i,j-1] + at[i,j]*y[i-1,j]
over the 14x14 grid (per batch*head, per feature), solved with row-wise
prefix scans (tensor_tensor_scan) on the vector engine.

v4 (this file): fp16 data plane.
  - x is converted to fp16 on the host (rel err ~2.4e-4, gate is 2e-2) and
    shipped transposed to (row, feat, col); y and out are fp16 on device and
    converted back on the host. DMA bytes halve in both directions.
  - The serial chain stays on the DVE in ONE chain (scan -> mul -> add
    back-to-back on the same engine, no cross-engine latency): fp16 gets the
    DVE 2x_1p mode for mul/add (measured 515/536 ns per 896-elem row vs 1086
    fp32); the scan runs at 2 cyc/elem regardless of dtype (1931 ns).
  - The out = y + D*x stage stays on the DVE as fp16 adds (~0.4 ns/elem,
    ~5.8us total). Offloading it to GpSimd was tried and REVERTED: DVE and
    GpSimd share SBUF ports, and a concurrent GpSimd tensor op slows
    in-flight DVE chain ops ~9x (measured 620ns -> 5557ns). D==1 (the
    harness always generates D=ones) takes a fused y+x add; general D
    pre-scales x on the Scalar engine.
  - The Sigmoid activation table load (1283 ns) is pulled to t=0 via a dummy
    activation so it overlaps the input DMA ramp.

Sharding: data-parallel over batch B=32 -> 4 batches/core on 8 cores.
"""

import numpy as np

import concourse.bass as bass
import concourse.bacc as bacc
import concourse.mybir as mybir
from concourse.tile import TileContext
from concourse.bass_utils import run_bass_kernel_spmd

F32 = mybir.dt.float32
F16 = mybir.dt.float16

HG, WG = 14, 14          # grid
L = HG * WG              # 196 nodes
B, NH, P = 32, 24, 64    # batch, heads, headdim
NCORES = 8
BLOC = B // NCORES       # 4 batches per core
NPART = BLOC * NH        # 96 partitions (b,h) pairs
ROW = P * WG             # 896 elements per grid row per partition
TOT = L * P              # 12544 elements per partition
INVERSE_FACTOR = 0.95

# row groups for the out stage / output DMA chunks (first group early so
# output DMA starts overlapping; small last group so the final store doesn't
# sit on the kernel tail)
OUT_GROUPS = [(0, 1), (2, 4), (5, 8), (9, 11), (12, 12), (13, 13)]
RAW_SCAN = True  # kept for test.py compat

_CACHE = {}


def _host_tables():
    nie = 2.0 * np.ones((HG, WG))
    nie[:, 0] -= 1.0
    nie[0, :] -= 1.0
    nie[nie < 1e-6] = 1.0
    norm = (INVERSE_FACTOR / np.sqrt(nie)).astype(np.float32)
    mask_l = np.ones((HG, WG), np.float32)
    mask_l[:, 0] = 0.0
    mask_t = np.ones((HG, WG), np.float32)
    mask_t[0, :] = 0.0
    tab = np.concatenate([(norm * mask_l).ravel(), (norm * mask_t).ravel()])
    return tab.astype(np.float32)  # [392]


def _raw_scan(nc, out, data0, data1):
    """tensor_tensor_scan with multi-dim APs (state = data0*state + data1,
    fp32 state, chained in AP iteration order; masked coefficients al[:,0]=0
    reset the chain at each feature boundary)."""
    eng = nc.vector
    ins = mybir.InstTensorScalarPtr(
        name=nc.get_next_instruction_name(),
        is_tensor_tensor_scan=True,
        is_scalar_tensor_tensor=True,
        op0=mybir.AluOpType.mult,
        op1=mybir.AluOpType.add,
        ins=[eng.lower_ap(data0),
             eng.lower_ap_or_imm(0.0),
             eng.lower_ap(data1)],
        outs=[eng.lower_ap(out)],
    )
    return eng.add_instruction(ins)


def _build_program(d_is_one=True):
    nc = bacc.Bacc("TRN2", target_bir_lowering=False, debug=False,
                   num_devices=NCORES)
    xin = nc.dram_tensor("xin", [NPART, TOT], F16, kind="ExternalInput")
    # dt | table | -bias | D  packed in one fp32 tensor -> single input DMA
    mscin = nc.dram_tensor("mscin", [NPART, 4 * L + 2], F32,
                           kind="ExternalInput")
    yout = nc.dram_tensor("yout", [NPART, TOT], F16, kind="ExternalOutput")

    Copy = mybir.ActivationFunctionType.Copy
    Sigmoid = mybir.ActivationFunctionType.Sigmoid

    with TileContext(nc) as tc:
        with tc.tile_pool(name="main", bufs=1) as pool, \
             tc.tile_pool(name="rowtmp", bufs=3) as rpool:
            xt = pool.tile([NPART, TOT], F16)     # x, (i, f, j) layout
            yt = pool.tile([NPART, TOT], F16)     # y
            ot = pool.tile([NPART, TOT], F16)     # out
            xd = pool.tile([NPART, TOT], F16) if not d_is_one else None
            msc = pool.tile([NPART, 4 * L + 2], F32)
            coeff = pool.tile([NPART, 2 * L], F32)
            c16 = pool.tile([NPART, 2 * L], F16)
            warm = pool.tile([NPART, 1], F32)
            dtt = msc[:, 0:2 * L]
            tab = msc[:, 2 * L:4 * L]
            nb = msc[:, 4 * L:4 * L + 1]
            dp = msc[:, 4 * L + 1:4 * L + 2]

            # pull the Sigmoid act-table load (1283ns) to t=0, before any
            # DMA lands
            nc.gpsimd.memset(warm[:, :], 0.0)
            nc.scalar.activation(out=warm[:, :], in_=warm[:, :], func=Sigmoid)

            # coefficient inputs first (sigmoid is the longest dep chain),
            # then the first rows of x, then the rest
            nc.sync.dma_start(out=msc[:, :], in_=mscin[:, :])
            nc.sync.dma_start(out=xt[:, 0:2 * ROW], in_=xin[:, 0:2 * ROW])
            for r0, r1 in [(2, 4), (5, 8), (9, 13)]:
                nc.sync.dma_start(
                    out=xt[:, r0 * ROW:(r1 + 1) * ROW],
                    in_=xin[:, r0 * ROW:(r1 + 1) * ROW],
                )

            nc.scalar.activation(out=coeff[:, :], in_=dtt, func=Sigmoid,
                                 bias=nb, scale=-1.0)
            # fold the normalization table in and downcast to fp16
            nc.vector.tensor_mul(out=c16[:, :], in0=coeff[:, :], in1=tab)

            if not d_is_one:
                # general-D path: xd = D*x on the Scalar engine (idle),
                # per out-group so it overlaps the chain
                for g0, g1 in OUT_GROUPS:
                    nc.scalar.activation(
                        out=xd[:, g0 * ROW:(g1 + 1) * ROW],
                        in_=xt[:, g0 * ROW:(g1 + 1) * ROW],
                        func=Copy, scale=dp)

            def al_bc(i):
                return c16[:, i * WG:(i + 1) * WG].unsqueeze(1) \
                    .broadcast_to([NPART, P, WG])

            def at_bc(i):
                return c16[:, L + i * WG:L + (i + 1) * WG].unsqueeze(1) \
                    .broadcast_to([NPART, P, WG])

            def v3(t, off):
                return t[:, off:off + ROW].rearrange(
                    "p (f j) -> p f j", f=P, j=WG)

            # --- row recurrence: single chain, all three ops on the DVE
            # back-to-back (no cross-engine latency); out-stage on GpSimd ---
            for i in range(HG):
                if i == 0:
                    _raw_scan(nc, v3(yt, 0), al_bc(0), v3(xt, 0))
                else:
                    off = i * ROW
                    tt = rpool.tile([NPART, ROW], F16, tag="tt")
                    vt = rpool.tile([NPART, ROW], F16, tag="vt")
                    nc.vector.tensor_mul(
                        out=v3(tt, 0), in0=v3(yt, off - ROW), in1=at_bc(i))
                    nc.vector.tensor_add(
                        out=vt[:, :], in0=tt[:, :], in1=xt[:, off:off + ROW])
                    _raw_scan(nc, v3(yt, off), al_bc(i), v3(vt, 0))

                # out stage on the DVE too: fp16 2x adds (~0.4 ns/elem) and
                # NO cross-engine SBUF port contention (a concurrent GpSimd
                # tensor op slows in-flight DVE ops ~9x, measured)
                for g in OUT_GROUPS:
                    if i == g[1]:
                        o0, o1 = g[0] * ROW, (g[1] + 1) * ROW
                        nc.vector.tensor_add(
                            out=ot[:, o0:o1], in0=yt[:, o0:o1],
                            in1=(xt if d_is_one else xd)[:, o0:o1])
                        nc.sync.dma_start(
                            out=yout[:, o0:o1], in_=ot[:, o0:o1])

    nc.compile()
    return nc


def _get_program(d_is_one=True):
    key = ("nc", d_is_one)
    if key not in _CACHE:
        _CACHE[key] = _build_program(d_is_one)
    return _CACHE[key]


def make_in_maps(dt, dt_bias, x, D):
    """Host-side sharding + per-row (j,f)->(f,j) transpose + fp16 + tables."""
    dt = np.ascontiguousarray(np.asarray(dt, dtype=np.float32))
    dt_bias = np.asarray(dt_bias, dtype=np.float32)
    x = np.asarray(x, dtype=np.float32)
    D = np.asarray(D, dtype=np.float32)

    tab = _host_tables()
    tabin = np.ascontiguousarray(np.broadcast_to(tab, (NPART, 2 * L)))
    nb = np.ascontiguousarray(np.tile(-dt_bias, BLOC).reshape(NPART, 1))
    dp = np.ascontiguousarray(np.tile(D, BLOC).reshape(NPART, 1))

    # pre-transpose x: (b,h,i,j,f) -> (b,h,i,f,j), flat per (b,h), fp16
    xT = np.ascontiguousarray(
        x.reshape(B, NH, HG, WG, P).transpose(0, 1, 2, 4, 3)
        .reshape(B, NH, TOT)).astype(np.float16)

    in_maps = []
    for c in range(NCORES):
        bs = slice(c * BLOC, (c + 1) * BLOC)
        dtc = np.ascontiguousarray(
            dt[:, bs].reshape(2, BLOC, NH, L).transpose(1, 2, 0, 3)
            .reshape(NPART, 2 * L))
        xc = np.ascontiguousarray(xT[bs].reshape(NPART, TOT))
        in_maps.append({
            "xin": xc,
            "mscin": np.ascontiguousarray(
                np.concatenate([dtc, tabin, nb, dp], axis=1)),
        })
    return in_maps


def _gather(results):
    """[96, TOT] fp16 (i,f,j)-layout shards -> full fp32 [B,NH,L,P]."""
    outs = []
    for r in results:
        o = np.asarray(r["yout"], dtype=np.float32)
        o = o.reshape(BLOC, NH, HG, P, WG).transpose(0, 1, 2, 4, 3)
        outs.append(o.reshape(BLOC, NH, L, P))
    return np.ascontiguousarray(np.concatenate(outs, axis=0))


def kernel(dt, dt_bias, x, D):
    d_is_one = bool(np.allclose(np.asarray(D, np.float32), 1.0))
    nc = _get_program(d_is_one)
    in_maps = make_in_maps(dt, dt_bias, x, D)
    res = run_bass_kernel_spmd(nc, in_maps, core_ids=list(range(NCORES)))
    return _gather(res.results)
